# revision 13
# baseline (speedup 1.0000x reference)
"""BertSelfAttention (softsign-modified) Trainium2 Bass kernel, v2.

Sharding: 8 cores = 2 batches x 4 head-groups (3 heads each).
Host gathers unmasked queries (mask applies along the QUERY dim only:
masked rows get uniform softmax => output = mean(V), filled host-side).

Device per core (fp16 matmuls, fp32 accumulation/softmax pipeline):
  - proj: qT/kT/vT = W_hT.T @ hiddenT (hiddenT streamed in slabs), fp16 out
  - k_mod = k/8 + k/(8+9|k|) + v  (algebraic refactor of
    k/8 + ss(ss(k)/8) + v; 6 DVE ops/chunk, fp16 throughout)
  - V natural layout via DMA xbar transpose (off the PE), ones col for
    sumexp kept from a gpsimd memset
  - scores^T[k,q] = km^T.T @ qT, two heads row-tiled concurrently
  - probs = exp(scores/8): ACT activation for most k-tiles, optional
    DVE Schraudolph (1 tensor_scalar: u16 bits = A*s + B) for a subset
  - ctx natural [q,65]: probs tiles stationary, [V|ones] moving;
    col 64 accumulates sumexp -> per-partition reciprocal normalize
  - unit22 (3rd head) ingredients packed as one [Wk_h2|Wv_h2] chain;
    v half shifted to partitions 0:63 by SBUF DMA, km duplicated to
    partitions 64:127 by SBUF DMA for the row-tiled pairing
  - emission interleaves unit01 ramp with the first attention block so
    ACT exp work starts as early as possible
"""

import functools
import os
import sys

import numpy as np

for _p in ("/opt/trn_rl_repo", "/root/.axon_site/_ro/trn_rl_repo"):
    if os.path.isdir(_p) and _p not in sys.path:
        sys.path.append(_p)

import concourse.bacc as bacc
import concourse.mybir as mybir
import concourse.tile as tile
from concourse import bass_utils

F32 = mybir.dt.float32
F16 = mybir.dt.float16
U16 = mybir.dt.uint16
ALU = mybir.AluOpType
ACTF = mybir.ActivationFunctionType

B, S, HD, H, D = 2, 4096, 768, 12, 64
NCORES = 8
HPC = 3  # heads per core
QB = 512  # q block (one PSUM bank of fp32)
KT = 128  # k tile (partition dim of scores^T)
NB = 512  # projection N block
KCH = HD // 128  # 6 contraction chunks
NKT = S // KT  # 32 k tiles
CH = 1024  # km chunk
SCALE = 0.125  # 1/sqrt(D)

# fp16 Schraudolph exp: u16 bits = EXPA * s + EXPB ~ fp16(exp(s/8))
EXPA = 184.6649652337873
EXPB = 15301.1

# Which k-tiles use the DVE exp (per full-width attention pass).
# kt % DVE_EXP_MOD == DVE_EXP_PHASE offloads 1/DVE_EXP_MOD of exp work.
DVE_EXP_MOD = 8

# bisect flags
NO_DVEEXP = bool(int(os.environ.get("KV2_NO_DVEEXP", "0")))
NO_SB2SB = bool(int(os.environ.get("KV2_NO_SB2SB", "0")))
NO_FUSE = bool(int(os.environ.get("KV2_NO_FUSE", "0")))
NO_XPOSE = bool(int(os.environ.get("KV2_NO_XPOSE", "0")))
TRUNC = int(os.environ.get("KV2_TRUNC", "99"))
U22MASK = int(os.environ.get("KV2_U22MASK", "63"))
SKIP_TAIL = bool(int(os.environ.get("KV2_SKIP_TAIL", "0")))


def _qblocks(P_q):
    """Split P_q into blocks: 512s then one optional 128..384 tail."""
    out = []
    q0 = 0
    while P_q - q0 >= QB:
        out.append((q0, QB))
        q0 += QB
    if P_q - q0:
        out.append((q0, P_q - q0))
    return out


def _emit(nc, tc, P_q, t):
    qbs = _qblocks(P_q)
    NG = P_q // 128

    with (
        tc.tile_pool(name="persist", bufs=1) as P,
        tc.tile_pool(name="work", bufs=2) as W,
        tc.tile_pool(name="scr", bufs=5) as SCR,
        tc.tile_pool(name="probs", bufs=3) as PRB,
        tc.tile_pool(name="psA", bufs=2, space="PSUM") as psA,
        tc.tile_pool(name="psB", bufs=2, space="PSUM") as psB,
        tc.tile_pool(name="psC", bufs=1, space="PSUM") as psC,
    ):
        # ---- persistent SBUF ----
        q01 = P.tile([128, P_q], F16)
        q22 = P.tile([128, P_q], F16)
        k01 = P.tile([128, S], F16)
        km01 = P.tile([128, S], F16)
        km22 = P.tile([128, S], F16)
        v01 = P.tile([128, S], F16)
        kv22 = P.tile([128, S], F16)  # [k_h2 (p 0:64) | v_h2 (p 64:128)]
        vsh22 = P.tile([64, S], F16)  # v_h2 shifted to partitions 0:64
        vn0 = P.tile([128, 65 * NKT], F16)  # V natural + ones col, head 0
        vn1 = P.tile([128, 65 * NKT], F16)
        vn2 = P.tile([128, 65 * NKT], F16)
        outst = P.tile([128, NG * 192], F32)
        mvsb = P.tile([1, 192], F32)

        wsb = {}
        bsb = {}
        for nm in ("q01", "q22", "k01", "v01", "kv22"):
            wsb[nm] = P.tile([128, KCH * 128], F16, name=f"w_{nm}_sb")
            nc.sync.dma_start(
                wsb[nm][:].rearrange("p (c m) -> p c m", c=KCH),
                t[f"w_{nm}"][:].rearrange("(c p) m -> p c m", p=128),
            )
            bsb[nm] = P.tile([128, 1], F32, name=f"b_{nm}_sb")
            nc.sync.dma_start(bsb[nm][:], t[f"b_{nm}"][:])

        def proj_block(src_ap, blk, chains, on_act):
            n0, w = blk
            slab = W.tile([128, KCH * NB], F16, tag="slab", name="slab")
            nc.sync.dma_start(
                slab[:, 0 : KCH * w].rearrange("p (c s) -> p c s", c=KCH),
                src_ap[:, n0 : n0 + w].rearrange("(c p) s -> p c s", p=128),
            )
            for nm, dst in chains:
                ps = psB.tile([128, NB], F32, tag="pp", name="pp")
                for c in range(KCH):
                    nc.tensor.matmul(
                        ps[:, 0:w],
                        wsb[nm][:, c * 128 : (c + 1) * 128],
                        slab[:, c * w : (c + 1) * w],
                        start=(c == 0),
                        stop=(c == KCH - 1),
                    )
                dsl = dst[:, n0 : n0 + w]
                if on_act:
                    nc.scalar.activation(
                        dsl, ps[:, 0:w], ACTF.Identity, bias=bsb[nm][:]
                    )
                else:
                    nc.vector.tensor_scalar_add(dsl, ps[:, 0:w], bsb[nm][:])

        def emit_km_chunk(kb, vb, kmb, ch):
            """km = k/8 + k/(8+9|k|) + v on partitions of kb/vb/kmb (fp16)."""
            sl = slice(ch * CH, (ch + 1) * CH)
            a = SCR.tile([128, CH], F16, tag="scr", name="a")
            npart = kb[1]
            kbs = kb[0][0:npart, sl]
            a_ = a[0:npart, 0:CH]
            nc.vector.tensor_scalar(
                a_.bitcast(U16), kbs.bitcast(U16), 0x7FFF, None,
                op0=ALU.bitwise_and,
            )
            d = SCR.tile([128, CH], F32, tag="scr32", name="d")
            nc.vector.tensor_scalar(
                d[0:npart, 0:CH], a_, 9.0, 8.0, op0=ALU.mult, op1=ALU.add
            )
            r = SCR.tile([128, CH], F32, tag="scr32", name="r")
            nc.vector.reciprocal_approx_fast(r[0:npart, 0:CH], d[0:npart, 0:CH])
            w_ = SCR.tile([128, CH], F16, tag="scr", name="w_")
            nc.vector.tensor_scalar_add(w_[0:npart, 0:CH], r[0:npart, 0:CH], SCALE)
            tt = SCR.tile([128, CH], F16, tag="scr", name="tt")
            nc.vector.tensor_mul(tt[0:npart, 0:CH], kbs, w_[0:npart, 0:CH])
            nc.vector.tensor_add(kmb[0:npart, sl], tt[0:npart, 0:CH], vb[0][0:npart, sl])

        def vnat_dma(vtile, p0, ch, vn):
            """DMA-xbar transpose v[p0:p0+64, ch*CH:(ch+1)*CH] -> vn k-tiles.

            The xbar writes transposed 128-col src tiles at a fixed pitch of
            64 output columns (dst AP strides are ignored on HW), so go via a
            contiguous scratch and DVE-copy each k-tile into the 65-stride
            [V|ones] layout."""
            scr = W.tile([128, 8 * 64], F16, tag="vscr", name="vscr")
            if NO_XPOSE:
                nc.vector.memset(scr[:], 0.5)
            else:
                nc.sync.dma_start_transpose(
                    scr[:].rearrange("p (t c) -> p t c", c=64),
                    vtile[p0 : p0 + 64, ch * CH : (ch + 1) * CH],
                )
            for tt in range(8):
                nc.vector.tensor_copy(
                    vn[:, (ch * 8 + tt) * 65 : (ch * 8 + tt) * 65 + 64],
                    scr[:, tt * 64 : (tt + 1) * 64],
                )

        # ---- attention ----
        def epilogue(ctx, w, col0, q0):
            """ctx: PSUM [128, (w//128)*65] natural layout, col 64 = sumexp."""
            for j in range(w // 128):
                rc = W.tile([128, 1], F32, tag="rc", name="rc")
                nc.vector.reciprocal(rc[:], ctx[:, j * 65 + 64 : j * 65 + 65])
                g = q0 // 128 + j
                nc.vector.tensor_scalar_mul(
                    outst[:, g * 192 + col0 : g * 192 + col0 + 64],
                    ctx[:, j * 65 : j * 65 + 64],
                    rc[:],
                )

        def attn_open(tag0, tag1):
            ctx0 = psC.tile([128, (QB // 128) * 65], F32, tag=tag0, name="ctx0")
            ctx1 = psC.tile([128, (QB // 128) * 65], F32, tag=tag1, name="ctx1")
            return ctx0, ctx1

        def attn_steps(ctx0, ctx1, kmbuf, qbuf, blkA, blkB, vnA, vnB, kts,
                       dve_phase=None):
            qa, wa = blkA
            qb_, wb = blkB
            for kt in kts:
                sc = psA.tile([128, 2 * QB], F32, tag="sc", name="sc")
                nc.tensor.matmul(
                    sc[:, 0:wa],
                    kmbuf[0:64, kt * KT : (kt + 1) * KT],
                    qbuf[0:64, qa : qa + wa],
                    start=True,
                    stop=True,
                )
                nc.tensor.matmul(
                    sc[:, QB : QB + wb],
                    kmbuf[64:128, kt * KT : (kt + 1) * KT],
                    qbuf[64:128, qb_ : qb_ + wb],
                    start=True,
                    stop=True,
                )
                pb = PRB.tile([128, 2 * QB], F16, tag="pb", name="pb")
                use_dve = ((not NO_DVEEXP) and dve_phase is not None
                           and kt % DVE_EXP_MOD == dve_phase and wa == QB)
                if use_dve:
                    nc.vector.tensor_scalar(
                        pb[:, 0 : QB + wb].bitcast(U16),
                        sc[:, 0 : QB + wb],
                        EXPA,
                        EXPB,
                        op0=ALU.mult,
                        op1=ALU.add,
                    )
                elif wa == QB:
                    nc.scalar.activation(
                        pb[:, 0 : QB + wb], sc[:, 0 : QB + wb], ACTF.Exp,
                        scale=SCALE,
                    )
                else:
                    nc.scalar.activation(
                        pb[:, 0:wa], sc[:, 0:wa], ACTF.Exp, scale=SCALE
                    )
                    nc.scalar.activation(
                        pb[:, QB : QB + wb], sc[:, QB : QB + wb], ACTF.Exp,
                        scale=SCALE,
                    )
                for j in range(wa // 128):
                    nc.tensor.matmul(
                        ctx0[:, j * 65 : (j + 1) * 65],
                        pb[:, j * 128 : (j + 1) * 128],
                        vnA[:, kt * 65 : kt * 65 + 65],
                        start=(kt == 0 and j == 0),
                        stop=(kt == NKT - 1 and j == wa // 128 - 1),
                    )
                for j in range(wb // 128):
                    nc.tensor.matmul(
                        ctx1[:, j * 65 : (j + 1) * 65],
                        pb[:, QB + j * 128 : QB + (j + 1) * 128],
                        vnB[:, kt * 65 : kt * 65 + 65],
                        start=(kt == 0 and j == 0),
                        stop=(kt == NKT - 1 and j == wb // 128 - 1),
                    )

        def attn_block(kmbuf, qbuf, blkA, blkB, vnA, vnB, colA, colB,
                       dve_phase=None):
            ctx0, ctx1 = attn_open("ctx0", "ctx1")
            attn_steps(ctx0, ctx1, kmbuf, qbuf, blkA, blkB, vnA, vnB,
                       range(NKT), dve_phase)
            epilogue(ctx0, blkA[1], colA, blkA[0])
            epilogue(ctx1, blkB[1], colB, blkB[0])

        def attn_tail(kmbuf, qbuf, blk, vn, col0):
            """Single q block, k tiles processed in row-tiled pairs."""
            qt, wt = blk
            ctx0 = psC.tile([128, (QB // 128) * 65], F32, tag="ctx0", name="ctxT")
            for k2 in range(NKT // 2):
                ka, kb = 2 * k2, 2 * k2 + 1
                sc = psA.tile([128, 2 * QB], F32, tag="sc", name="sc")
                nc.tensor.matmul(
                    sc[:, 0:wt],
                    kmbuf[0:64, ka * KT : (ka + 1) * KT],
                    qbuf[0:64, qt : qt + wt],
                    start=True,
                    stop=True,
                )
                nc.tensor.matmul(
                    sc[:, QB : QB + wt],
                    kmbuf[64:128, kb * KT : (kb + 1) * KT],
                    qbuf[64:128, qt : qt + wt],
                    start=True,
                    stop=True,
                )
                pb = PRB.tile([128, 2 * QB], F16, tag="pb", name="pb")
                nc.scalar.activation(
                    pb[:, 0:wt], sc[:, 0:wt], ACTF.Exp, scale=SCALE
                )
                nc.scalar.activation(
                    pb[:, QB : QB + wt], sc[:, QB : QB + wt], ACTF.Exp,
                    scale=SCALE,
                )
                for j in range(wt // 128):
                    nc.tensor.matmul(
                        ctx0[:, j * 65 : (j + 1) * 65],
                        pb[:, j * 128 : (j + 1) * 128],
                        vn[:, ka * 65 : ka * 65 + 65],
                        start=(k2 == 0 and j == 0),
                        stop=False,
                    )
                    nc.tensor.matmul(
                        ctx0[:, j * 65 : (j + 1) * 65],
                        pb[:, QB + j * 128 : QB + (j + 1) * 128],
                        vn[:, kb * 65 : kb * 65 + 65],
                        start=False,
                        stop=(k2 == NKT // 2 - 1 and j == wt // 128 - 1),
                    )
            epilogue(ctx0, wt, col0, qt)

        # ================= emission =================
        for vn in (vn0, vn1, vn2):
            nc.gpsimd.memset(vn[:], 1.0)
        if TRUNC < 2:
            return

        # -- unit01 ramp fused with first attention block --
        # ch loop: proj 2 blocks -> km chunk -> vnat transposes; from ch>=1
        # also run the first attention block's k-tiles for ready chunks.
        blk0 = qbs[0]
        if NO_FUSE:
            for ch in range(S // CH):
                proj_block(t["hT_full"], (ch * CH, NB), [("k01", k01), ("v01", v01)],
                           on_act=True)
                proj_block(t["hT_full"], (ch * CH + NB, NB),
                           [("k01", k01), ("v01", v01)], on_act=True)
                emit_km_chunk((k01, 128), (v01,), km01, ch)
                vnat_dma(v01, 0, ch, vn0)
                vnat_dma(v01, 64, ch, vn1)
            for blk in qbs:
                proj_block(t["hT_sel"], blk, [("q01", q01)], on_act=True)
            attn_block(km01, q01, blk0, blk0, vn0, vn1, 0, 64, dve_phase=1)
        else:
            ctxE0, ctxE1 = attn_open("ctx0", "ctx1")
            for ch in range(S // CH):
                proj_block(t["hT_full"], (ch * CH, NB), [("k01", k01), ("v01", v01)],
                           on_act=True)
                proj_block(t["hT_full"], (ch * CH + NB, NB),
                           [("k01", k01), ("v01", v01)], on_act=True)
                emit_km_chunk((k01, 128), (v01,), km01, ch)
                vnat_dma(v01, 0, ch, vn0)
                vnat_dma(v01, 64, ch, vn1)
                if ch == 0:
                    proj_block(t["hT_sel"], blk0, [("q01", q01)], on_act=True)
                else:
                    attn_steps(ctxE0, ctxE1, km01, q01, blk0, blk0, vn0, vn1,
                               range(8 * (ch - 1), 8 * ch), dve_phase=1)
            for blk in qbs[1:]:
                proj_block(t["hT_sel"], blk, [("q01", q01)], on_act=True)
            attn_steps(ctxE0, ctxE1, km01, q01, blk0, blk0, vn0, vn1,
                       range(8 * (S // CH - 1), NKT), dve_phase=1)
            epilogue(ctxE0, blk0[1], 0, blk0[0])
            epilogue(ctxE1, blk0[1], 64, blk0[0])

        if TRUNC < 3:
            return
        # -- unit22 ingredient slices, interleaved into unit01 attention --
        def u22_slice(i):
            if U22MASK & 1:
                proj_block(t["hT_full"], (i * CH, NB), [("kv22", kv22)], on_act=False)
                proj_block(t["hT_full"], (i * CH + NB, NB), [("kv22", kv22)],
                           on_act=False)
            sl = slice(i * CH, (i + 1) * CH)
            if U22MASK & 2:
                if NO_SB2SB:
                    nc.sync.dma_start(t["scr_dram"][:, 0:CH], kv22[64:128, sl])
                    nc.sync.dma_start(vsh22[:, sl], t["scr_dram"][:, 0:CH])
                else:
                    nc.sync.dma_start(vsh22[:, sl], kv22[64:128, sl])
            if U22MASK & 4:
                emit_km_chunk((kv22, 64), (vsh22,), km22, i)
            if U22MASK & 8:
                if NO_SB2SB:
                    nc.sync.dma_start(t["scr_dram"][:, CH : 2 * CH], km22[0:64, sl])
                    nc.sync.dma_start(km22[64:128, sl], t["scr_dram"][:, CH : 2 * CH])
                else:
                    nc.sync.dma_start(km22[64:128, sl], km22[0:64, sl])
            if U22MASK & 16:
                vnat_dma(kv22, 64, i, vn2)

        nq22 = 0

        def q22_some(n):
            nonlocal nq22
            if U22MASK & 32:
                for blk in qbs[nq22 : nq22 + n]:
                    proj_block(t["hT_sel"], blk, [("q22", q22)], on_act=False)
            nq22 += n

        # unit01 attention for remaining q blocks with u22 work interleaved
        for qi, blk in enumerate(qbs[1:]):
            if qi < S // CH:
                u22_slice(qi)
                q22_some(1 if qi > 0 else 2)
            if SKIP_TAIL and blk[1] != QB:
                continue
            attn_block(km01, q01, blk, blk, vn0, vn1, 0, 64, dve_phase=1)
        for i in range(len(qbs) - 1, S // CH):
            u22_slice(i)
        q22_some(len(qbs) - nq22)

        if TRUNC < 4:
            return
        # ---- meanV row ----
        for h, vn in ((0, vn0), (1, vn1), (2, vn2)):
            mv = psB.tile([1, 64], F32, tag="pp", name="mv")
            for kt in range(NKT):
                nc.tensor.matmul(
                    mv[:],
                    vn[:, kt * 65 + 64 : kt * 65 + 65],
                    vn[:, kt * 65 : kt * 65 + 64],
                    start=(kt == 0),
                    stop=(kt == NKT - 1),
                )
            nc.vector.tensor_scalar_mul(mvsb[:, h * 64 : (h + 1) * 64], mv[:], 1.0 / S)
        nc.sync.dma_start(t["out"][P_q : P_q + 1, :], mvsb[:])

        # unit22: head 2 self-paired across q blocks
        done_g = 0

        def store_groups(hi):
            nonlocal done_g
            for g in range(done_g, hi):
                nc.sync.dma_start(
                    t["out"][g * 128 : (g + 1) * 128, :],
                    outst[:, g * 192 : (g + 1) * 192],
                )
            done_g = hi

        if TRUNC < 5:
            return
        for st in range(len(qbs) // 2):
            bA, bB = qbs[2 * st], qbs[2 * st + 1]
            attn_block(km22, q22, bA, bB, vn2, vn2, 128, 128, dve_phase=3)
            store_groups((bB[0] + bB[1]) // 128)
        if len(qbs) % 2:
            attn_tail(km22, q22, qbs[-1], vn2, 128)
        store_groups(NG)


@functools.lru_cache(maxsize=4)
def _build(P_q):
    nc = bacc.Bacc(
        "TRN2",
        target_bir_lowering=False,
        debug=False,
        enable_asserts=False,
        num_devices=NCORES,
    )
    t = {}
    t["hT_full"] = nc.dram_tensor("hT_full", [HD, S], F16, kind="ExternalInput").ap()
    t["hT_sel"] = nc.dram_tensor("hT_sel", [HD, P_q], F16, kind="ExternalInput").ap()
    for nm in ("q01", "q22", "k01", "v01", "kv22"):
        t[f"w_{nm}"] = nc.dram_tensor(
            f"w_{nm}", [HD, 128], F16, kind="ExternalInput"
        ).ap()
        t[f"b_{nm}"] = nc.dram_tensor(
            f"b_{nm}", [128, 1], F32, kind="ExternalInput"
        ).ap()
    t["out"] = nc.dram_tensor("out", [P_q + 1, 192], F32, kind="ExternalOutput").ap()
    t["scr_dram"] = nc.dram_tensor("scr_dram", [64, 2 * CH], F16, kind="Internal").ap()

    with tile.TileContext(nc) as tc:
        _emit(nc, tc, P_q, t)
    nc.compile()
    return nc


def _prep_core_inputs(hidden, sel_pad, Wq, bq, Wk, bk, Wv, bv, heads):
    """Build the in_map for one core. hidden: [S, HD] for this batch."""
    h0, h1, h2 = heads
    m = {}
    m["hT_full"] = np.ascontiguousarray(hidden.T.astype(np.float16))
    m["hT_sel"] = np.ascontiguousarray(hidden[sel_pad].T.astype(np.float16))

    def wT(Wmat, h):
        return np.ascontiguousarray(Wmat[h * D : (h + 1) * D, :].T)

    def bs(bvec, h):
        return bvec[h * D : (h + 1) * D]

    for nm, Wmat, bvec in (("q", Wq, bq), ("k", Wk, bk), ("v", Wv, bv)):
        if nm in ("q",):
            m["w_q01"] = np.concatenate([wT(Wmat, h0), wT(Wmat, h1)], axis=1)
            m["w_q22"] = np.concatenate([wT(Wmat, h2), wT(Wmat, h2)], axis=1)
            m["b_q01"] = np.concatenate([bs(bvec, h0), bs(bvec, h1)]).reshape(128, 1)
            m["b_q22"] = np.concatenate([bs(bvec, h2), bs(bvec, h2)]).reshape(128, 1)
    m["w_k01"] = np.concatenate([wT(Wk, h0), wT(Wk, h1)], axis=1)
    m["b_k01"] = np.concatenate([bs(bk, h0), bs(bk, h1)]).reshape(128, 1)
    m["w_v01"] = np.concatenate([wT(Wv, h0), wT(Wv, h1)], axis=1)
    m["b_v01"] = np.concatenate([bs(bv, h0), bs(bv, h1)]).reshape(128, 1)
    m["w_kv22"] = np.concatenate([wT(Wk, h2), wT(Wv, h2)], axis=1)
    m["b_kv22"] = np.concatenate([bs(bk, h2), bs(bv, h2)]).reshape(128, 1)
    for k in list(m):
        dt = np.float16 if (k.startswith("w_") or k.startswith("hT_")) else np.float32
        m[k] = np.ascontiguousarray(m[k], dtype=dt)
    return m


def _plan(attention_mask):
    """Returns (P_q, sel list, sel_pad list)."""
    sels = [np.where(attention_mask[b] != 0)[0] for b in range(B)]
    nmax = max(1, max(len(s) for s in sels))
    P_q = ((nmax + 127) // 128) * 128
    if P_q < 256:
        P_q = 256
    sel_pads = []
    for s in sels:
        pad = np.zeros(P_q, dtype=np.int64)
        pad[: len(s)] = s
        sel_pads.append(pad)
    return P_q, sels, sel_pads


def build_in_maps(hidden_states, attention_mask, Wq, bq, Wk, bk, Wv, bv):
    P_q, sels, sel_pads = _plan(np.asarray(attention_mask))
    hs = np.asarray(hidden_states, dtype=np.float32)
    in_maps = []
    for c in range(NCORES):
        b, g = c // 4, c % 4
        heads = (3 * g, 3 * g + 1, 3 * g + 2)
        in_maps.append(
            _prep_core_inputs(hs[b], sel_pads[b], Wq, bq, Wk, bk, Wv, bv, heads)
        )
    return P_q, sels, in_maps


def assemble(results, P_q, sels, attention_mask):
    out = np.empty((B, S, HD), dtype=np.float32)
    mask = np.asarray(attention_mask)
    for c in range(NCORES):
        b, g = c // 4, c % 4
        r = results[c]["out"]
        cols = slice(192 * g, 192 * (g + 1))
        sel = sels[b]
        if len(sel):
            out[b, sel, cols] = r[: len(sel)]
        inv = np.where(mask[b] == 0)[0]
        if len(inv):
            out[b, inv, cols] = r[P_q]
    return out


def _install_ntff_shim():
    """Provide antenv.axon_hooks (missing from this image) so
    run_bass_kernel_spmd(trace=True) can capture NTFF profiles, and stub
    out the network-dependent artifact upload."""
    import types

    try:
        import antenv
    except ImportError:
        return
    try:
        from antenv.axon_hooks import get_axon_ntff_profile_hook  # noqa: F401
    except ImportError:
        try:
            if "/root/.axon_site" not in sys.path:
                sys.path.insert(0, "/root/.axon_site")
            from trn_agent_boot.trn_boot import _ntff_profile_via_ctypes

            hook = _ntff_profile_via_ctypes("/opt/axon/libaxon_pjrt.so")
        except Exception:
            hook = None
        mod = types.ModuleType("antenv.axon_hooks")
        _h = {"h": hook}
        mod.get_axon_ntff_profile_hook = lambda: _h["h"]
        mod.set_axon_ntff_profile_hook = lambda h: _h.__setitem__("h", h)
        sys.modules["antenv.axon_hooks"] = mod
        antenv.axon_hooks = mod

    _orig_upload = bass_utils.upload_artifacts

    def _safe_upload(tmpdir):
        try:
            return _orig_upload(tmpdir)
        except Exception:
            return tmpdir

    bass_utils.upload_artifacts = _safe_upload


def kernel(hidden_states, attention_mask, Wq, bq, Wk, bk, Wv, bv, trace=False):
    if trace:
        _install_ntff_shim()
    P_q, sels, in_maps = build_in_maps(
        hidden_states, attention_mask, Wq, bq, Wk, bk, Wv, bv
    )
    nc = _build(P_q)
    res = bass_utils.run_bass_kernel_spmd(
        nc, in_maps, core_ids=list(range(NCORES)), trace=trace
    )
    out = assemble(res.results, P_q, sels, attention_mask)
    if trace:
        kernel.last_exec_time_ns = res.exec_time_ns
        kernel.last_results = res
    return out
